# revision 19
# baseline (speedup 1.0000x reference)
"""Causal multi-head attention block (qkv proj + partial RoPE + causal attn +
out proj) for Trainium2, distributed over 8 NeuronCores.

Sharding: core i handles batch b = i//2 and head-group g = i%2 (6 of 12 heads).
Each core computes a partial output projection (contraction over its 6 heads'
384 channels); the host sums the two head-group partials per batch.

v4 design notes (over v3):
  - All matmul operands bf16 (separate pipelined LDWEIGHTS + FWL; fp32r
    fuses a serial 4-byte weight load into every matmul).
  - DMA coalescing: the Sync sequencer issues each DMA in ~600ns serially,
    and v3 had ~43 DMAs per tile (plus 25 at startup) — the issue stream
    alone stalled the pipeline.  v4: weights = 1 DMA each, x = 1/tile,
    rope scatter = 2/tile, pass scatter = 1/tile, rowsums = 3/tile,
    out = 1/tile.  Enabled by reordering wqk rows into (block, head-parity,
    dim) groups of 96 so every scatter is one affine access pattern.
  - Deferred finish: softmax normalization (rowsum DMA -> reciprocal ->
    broadcast matmul -> multiply) and the out-projection of tile jt are
    emitted AFTER tile jt+1's projection matmuls, so the serial chain
    never blocks the strict-FIFO PE queue (v3 lost ~6us per tile there).
  - PSUM->SBUF evictions of projection tiles moved to ScalarE (idle during
    the projection phase; DVE was becoming the second bottleneck).
  - Causal diag mask as one fused gpsimd op over both heads.
"""

import numpy as np

B, T, C = 4, 2048, 768
NH, HD, RD = 12, 64, 16
NHL = NH // 2          # heads per core (local)
NPAIR = NHL // 2       # head pairs per core
CL = NHL * HD          # local channels (384)
TQ = 512               # q tile
NTQ = T // TQ
NKT = T // 128         # k tiles of 128

_cache = {}


def _build():
    import concourse.bacc as bacc
    import concourse.mybir as mybir
    import concourse.tile as tile

    F32 = mybir.dt.float32
    BF16 = mybir.dt.bfloat16
    AF = mybir.ActivationFunctionType
    MUL = mybir.AluOpType.mult
    SUB = mybir.AluOpType.subtract
    ADD = mybir.AluOpType.add

    nc = bacc.Bacc(trn_type="TRN2", name="attn8")

    xt = nc.dram_tensor("xt", [C, T], BF16, kind="ExternalInput")
    wqkt = nc.dram_tensor("wqkt", [C, 2 * CL], BF16, kind="ExternalInput")
    wvt = nc.dram_tensor("wvt", [C, CL], BF16, kind="ExternalInput")
    wot = nc.dram_tensor("wot", [CL, C], BF16, kind="ExternalInput")
    cosb = nc.dram_tensor("cosb", [96, T], BF16, kind="ExternalInput")
    sinb = nc.dram_tensor("sinb", [96, T], BF16, kind="ExternalInput")
    tri = nc.dram_tensor("tri", [128, 2, 128], BF16, kind="ExternalInput")
    e6 = nc.dram_tensor("e6", [6, NPAIR * 128], BF16, kind="ExternalInput")
    out = nc.dram_tensor("out", [C, T], BF16, kind="ExternalOutput")

    # qk-projection M-tiles (wqkt column order, host-built), all 96 rows:
    #   tile 0: r1 rows (g, hh, d8)  g=block 0..5 (q p0..p2, k p0..p2),
    #           hh=head parity, d=rope dim 0:8
    #   tile 1: r2 rows (g, hh, d8)  rope dims 8:16
    #   tiles 2..7: pass tile g: (hh, d48) = block g's dims 16:64
    with tile.TileContext(nc) as tc:
        with (
            tc.tile_pool(name="persist", bufs=1) as pp,
            tc.tile_pool(name="weights", bufs=1) as wp,
            tc.tile_pool(name="xload", bufs=2) as xlp,
            tc.tile_pool(name="pstage", bufs=2) as psg,
            tc.tile_pool(name="ropet", bufs=1) as rtp,
            tc.tile_pool(name="expp", bufs=3) as xpp,
            tc.tile_pool(name="misc", bufs=2) as msc,
            tc.tile_pool(name="onorm", bufs=6) as onp,
            tc.tile_pool(name="rsp", bufs=2) as rsp,
            tc.tile_pool(name="flex", bufs=2, space="PSUM") as flx,
            tc.tile_pool(name="sps", bufs=2, space="PSUM") as sps,
            tc.tile_pool(name="ops", bufs=1, space="PSUM") as ops,
        ):
            qk_sb = pp.tile([128, 2 * NPAIR, T], BF16, tag="qk")
            v_sb = pp.tile([128, NKT, NHL, HD + 1], BF16, tag="v")
            o_sb = pp.tile([128, NPAIR, T], BF16, tag="o")
            cos_t = pp.tile([96, T], BF16, tag="cos")
            sin_t = pp.tile([96, T], BF16, tag="sin")
            tri_t = pp.tile([128, 2, 128], BF16, tag="tri")
            e6_t = pp.tile([6, NPAIR * 128], BF16, tag="e6")
            rot1 = pp.tile([96, T], BF16, tag="rot1")
            rot2 = pp.tile([96, T], BF16, tag="rot2")

            wqk_t = wp.tile([128, C // 128, 2 * CL], BF16, tag="wqk")
            wv_t = wp.tile([128, C // 128, CL], BF16, tag="wv")
            wo_t = wp.tile([128, NPAIR, C], BF16, tag="wo")

            qk_v = qk_sb.rearrange("(hh p) b t -> hh p b t", hh=2)

            # startup order: x(0) + rope weight columns first so the first
            # projection matmuls start as early as possible
            x_tiles = {}
            x_tiles[0] = xlp.tile([128, C // 128, TQ], BF16, tag="x", name="x0")
            nc.sync.dma_start(
                x_tiles[0],
                xt.rearrange("(co p) t -> p co t", p=128)[:, :, 0:TQ])
            wqk_r = wqkt.rearrange("(co p) m -> p co m", p=128)
            nc.sync.dma_start(wqk_t[:, :, 0:192], wqk_r[:, :, 0:192])
            nc.sync.dma_start(cos_t, cosb[:, :])
            nc.sync.dma_start(sin_t, sinb[:, :])
            nc.sync.dma_start(wqk_t[:, :, 192:2 * CL], wqk_r[:, :, 192:2 * CL])
            nc.sync.dma_start(wv_t, wvt.rearrange("(co p) m -> p co m", p=128))
            nc.sync.dma_start(tri_t, tri[:, :, :])
            nc.gpsimd.memset(
                v_sb.bitcast(mybir.dt.uint16).rearrange("p a b c -> p (a b c)"),
                0x3F80)  # bf16 1.0 bit pattern
            # needed first at finish(0), well after startup
            nc.sync.dma_start(wo_t, wot.rearrange("(po p) m -> p po m", p=128))
            nc.sync.dma_start(e6_t, e6[:, :])

            state = {}  # per-jt tiles needed by the deferred finish

            def proj(jt):
                ts = slice(jt * TQ, (jt + 1) * TQ)
                x_jt = x_tiles[jt]
                # rope M-tiles (r1, r2) into one 2-bank psum tile
                ps_r = sps.tile([128, 2, TQ], F32, tag="s")
                for mt in range(2):
                    for c in range(C // 128):
                        nc.tensor.matmul(
                            ps_r[0:96, mt, :],
                            wqk_t[:, c, 96 * mt:96 * (mt + 1)],
                            x_jt[:, c], start=(c == 0), stop=(c == C // 128 - 1))
                # rope: rot1 = r1*cos - r2*sin ; rot2 = r2*cos + r1*sin
                t1 = rtp.tile([96, TQ], F32, tag="t1")
                t2 = rtp.tile([96, TQ], F32, tag="t2")
                nc.vector.tensor_tensor(t1, ps_r[0:96, 0, :], cos_t[:, ts], MUL)
                nc.vector.tensor_tensor(t2, ps_r[0:96, 1, :], sin_t[:, ts], MUL)
                nc.vector.tensor_tensor(rot1[:, ts], t1, t2, SUB)
                t3 = rtp.tile([96, TQ], F32, tag="t1")
                t4 = rtp.tile([96, TQ], F32, tag="t2")
                nc.vector.tensor_tensor(t3, ps_r[0:96, 1, :], cos_t[:, ts], MUL)
                nc.vector.tensor_tensor(t4, ps_r[0:96, 0, :], sin_t[:, ts], MUL)
                nc.vector.tensor_tensor(rot2[:, ts], t3, t4, ADD)
                # rope scatter: rot rows are (hh, d, g), so the 48-row block of
                # one head parity linearizes exactly as the dst (d8, blk6, t)
                # iteration; one plain-slice DMA per (rot tile, head parity)
                for hh in range(2):
                    nc.sync.dma_start(qk_v[hh, 0:8, :, ts],
                                      rot1[48 * hh:48 * hh + 48, ts])
                    nc.sync.dma_start(qk_v[hh, 8:16, :, ts],
                                      rot2[48 * hh:48 * hh + 48, ts])

                # pass M-tiles: 6 tiles of 96 rows -> staged -> scatter DMA.
                # q tiles (g 0:3) first + their scatter issued early: the
                # attention phase needs q rows immediately, while k rows of
                # this tile are only needed by the 4 diagonal blocks at the
                # end of the kt loop.
                stg = psg.tile([96, C // 128, TQ], BF16, tag="pstg")
                for g in range(6):
                    ps = flx.tile([128, TQ], F32, tag="flex")
                    for c in range(C // 128):
                        nc.tensor.matmul(
                            ps[0:96], wqk_t[:, c, 192 + 96 * g:192 + 96 * (g + 1)],
                            x_jt[:, c], start=(c == 0), stop=(c == C // 128 - 1))
                    nc.scalar.copy(stg[:, g], ps[0:96])
                    if g == 2:
                        for hh in range(2):
                            nc.sync.dma_start(qk_v[hh, 16:64, 0:3, ts],
                                              stg[48 * hh:48 * hh + 48, 0:3, :])
                for hh in range(2):
                    nc.sync.dma_start(qk_v[hh, 16:64, 3:6, ts],
                                      stg[48 * hh:48 * hh + 48, 3:6, :])

                # v projection (x stationary -> [token, channel] layout)
                for vt in range(TQ // 128):
                    pvf = flx.tile([128, TQ], F32, tag="flex")
                    pv = pvf[:, 0:CL]
                    kt0 = jt * (TQ // 128) + vt
                    for c in range(C // 128):
                        nc.tensor.matmul(
                            pv, x_jt[:, c, vt * 128:(vt + 1) * 128],
                            wv_t[:, c], start=(c == 0), stop=(c == C // 128 - 1))
                    nc.scalar.copy(
                        v_sb[:, kt0, :, 0:HD],
                        pv.rearrange("p (h d) -> p h d", d=HD))

            def attention(jq):
                qs = slice(jq * TQ, (jq + 1) * TQ)
                rs6 = rsp.tile([6, TQ], F32, tag="rs6")
                ouns = []
                for p in range(NPAIR):
                    qb = qk_sb[:, p, qs]
                    kb = qk_sb[:, NPAIR + p, :]
                    o_ps = ops.tile([128, 2, TQ], F32, tag="o")
                    nkt = 4 * (jq + 1)
                    for kt in range(nkt):
                        m = kt - 4 * jq
                        a = 0 if m < 0 else 128 * m
                        ks = slice(kt * 128, (kt + 1) * 128)
                        sg = sps.tile([128, 2, TQ], F32, tag="s")
                        nc.tensor.matmul(
                            sg[:, 0, a:TQ], kb[0:64, ks], qb[0:64, a:TQ],
                            start=True, stop=True, tile_position=(0, 0))
                        nc.tensor.matmul(
                            sg[:, 1, a:TQ], kb[64:128, ks], qb[64:128, a:TQ],
                            start=True, stop=True, tile_position=(64, 0))
                        ep = xpp.tile([128, 2, TQ], BF16, tag="e")
                        nc.scalar.activation(ep[:, :, a:TQ], sg[:, :, a:TQ],
                                             AF.Exp, scale=0.125)
                        if m >= 0:
                            nc.gpsimd.tensor_tensor(
                                ep[:, :, a:a + 128],
                                ep[:, :, a:a + 128], tri_t, MUL)
                        for h in range(2):
                            nc.tensor.matmul(
                                o_ps[0:65, h, a:TQ],
                                v_sb[:, kt, 2 * p + h, :], ep[:, h, a:TQ],
                                start=(kt == 0), stop=(kt == nkt - 1))
                    oun = onp.tile([128, 2, TQ], F32, tag="oun")
                    nc.vector.tensor_copy(oun[0:65, :, :], o_ps[0:65, :, :])
                    for h in range(2):
                        nc.gpsimd.dma_start(rs6[2 * p + h:2 * p + h + 1, :],
                                            oun[64:65, h, :])
                    ouns.append(oun)
                state[jq] = (rs6, ouns)

            def finish_recip(jq):
                rs6, ouns = state[jq]
                rinv6 = rsp.tile([6, TQ], BF16, tag="rinv6")
                with nc.allow_low_precision(reason="softmax denom in bf16"):
                    nc.vector.reciprocal(rinv6, rs6)
                state[jq] = (rinv6, ouns)

            def finish(jq):
                qs = slice(jq * TQ, (jq + 1) * TQ)
                rinv6, ouns = state.pop(jq)
                for p in range(NPAIR):
                    bc = flx.tile([128, TQ], F32, tag="flex")
                    nc.tensor.matmul(bc, e6_t[:, p * 128:(p + 1) * 128], rinv6,
                                     start=True, stop=True)
                    oun = ouns[p]
                    with nc.allow_low_precision(reason="o normalize in bf16"):
                        nc.vector.tensor_tensor(
                            o_sb[0:64, p, qs], oun[0:64, 0, :], bc[0:64], MUL)
                        nc.vector.tensor_tensor(
                            o_sb[64:128, p, qs], oun[0:64, 1, :], bc[64:128], MUL)
                ost = msc.tile([128, C // 128, TQ], BF16, tag="ost")
                for dt in range(C // 128):
                    po = flx.tile([128, TQ], F32, tag="flex")
                    for p in range(NPAIR):
                        nc.tensor.matmul(
                            po, wo_t[:, p, dt * 128:(dt + 1) * 128],
                            o_sb[:, p, qs], start=(p == 0), stop=(p == NPAIR - 1))
                    nc.vector.tensor_copy(ost[:, dt], po)
                nc.sync.dma_start(
                    out.rearrange("(do p) t -> p do t", p=128)[:, :, qs], ost)

            for jt in range(NTQ):
                if jt >= 1:
                    finish_recip(jt - 1)
                proj(jt)
                if jt + 1 < NTQ:
                    x_tiles[jt + 1] = xlp.tile(
                        [128, C // 128, TQ], BF16, tag="x", name=f"x{jt + 1}")
                    nc.sync.dma_start(
                        x_tiles[jt + 1],
                        xt.rearrange("(co p) t -> p co t", p=128)[
                            :, :, (jt + 1) * TQ:(jt + 2) * TQ])
                attention(jt)
                # finish(jt-1) emitted after attention(jt): its reciprocal
                # chain then has the whole attention phase to complete, so
                # the broadcast matmul never stalls the PE queue
                if jt >= 1:
                    finish(jt - 1)
            finish_recip(NTQ - 1)
            finish(NTQ - 1)

    nc.compile()
    return nc


def _host_inputs(x, w_qkv, w_out):
    """Build per-core input dicts. Core i: batch i//2, head-group i%2."""
    import ml_dtypes

    BF = ml_dtypes.bfloat16
    xf = np.ascontiguousarray(x, dtype=np.float32)
    w3 = np.asarray(w_qkv, dtype=np.float32).reshape(3, NH, HD, C)
    wo = np.asarray(w_out, dtype=np.float32)

    per_group = []
    for g in range(2):
        h0 = g * NHL
        # rope tiles r1/r2 rows: (hh, d8, blk); blk 0..2 = q pairs, 3..5 = k
        r1, r2, passes = [], [], []
        for hh in range(2):
            for d in range(8):
                for blk in range(6):
                    tn = 0 if blk < 3 else 1
                    pr = blk % 3
                    h = h0 + 2 * pr + hh
                    r1.append(w3[tn, h, d:d + 1])
                    r2.append(w3[tn, h, 8 + d:9 + d])
        for blk in range(6):
            tn = 0 if blk < 3 else 1
            pr = blk % 3
            rows = []
            for hh in range(2):
                h = h0 + 2 * pr + hh
                rows.append(w3[tn, h, 16:64])
            passes.append(np.concatenate(rows, axis=0))    # [96, C]
        wqk = np.concatenate(
            [np.concatenate(r1, axis=0), np.concatenate(r2, axis=0)] + passes,
            axis=0)                                        # [768, C]
        wqkt = np.ascontiguousarray(wqk.T).astype(BF)      # [C, 768]
        wv = w3[2, h0:h0 + NHL].reshape(CL, C)             # [384, C]
        wvt = np.ascontiguousarray(wv.T).astype(BF)
        wotr = np.ascontiguousarray(wo[:, g * CL:(g + 1) * CL].T).astype(BF)
        per_group.append((wqkt, wvt, wotr))

    j = np.arange(RD // 2, dtype=np.float64)
    freqs = 1.0 / (10000.0 ** (2 * j / RD))
    t = np.arange(T, dtype=np.float64)
    ang = t[None, :] * freqs[:, None]                      # [8, T]
    # rows (hh, d, g): frequency d repeated for the 6 blocks, twice
    cosb = np.tile(np.repeat(np.cos(ang), 6, axis=0), (2, 1)).astype(BF)
    sinb = np.tile(np.repeat(np.sin(ang), 6, axis=0), (2, 1)).astype(BF)

    kk = np.arange(128)[:, None]
    qq = np.arange(128)[None, :]
    tri = np.broadcast_to(
        (kk <= qq)[:, None, :], (128, 2, 128)).astype(BF).copy()
    e6 = np.zeros((6, NPAIR * 128), dtype=np.float32)
    for p in range(NPAIR):
        e6[2 * p, p * 128:p * 128 + 64] = 1.0
        e6[2 * p + 1, p * 128 + 64:(p + 1) * 128] = 1.0
    e6 = e6.astype(BF)

    in_maps = []
    for i in range(8):
        b, g = divmod(i, 2)
        wqkt, wvt, wotr = per_group[g]
        in_maps.append({
            "xt": np.ascontiguousarray(xf[b].T).astype(BF),
            "wqkt": wqkt, "wvt": wvt, "wot": wotr,
            "cosb": cosb, "sinb": sinb, "tri": tri, "e6": e6,
        })
    return in_maps


def kernel(x, w_qkv, w_out, _trace=False):
    from concourse.bass_utils import run_bass_kernel_spmd

    if "nc" not in _cache:
        _cache["nc"] = _build()
    nc = _cache["nc"]
    in_maps = _host_inputs(x, w_qkv, w_out)
    res = run_bass_kernel_spmd(nc, in_maps, core_ids=list(range(8)),
                               trace=_trace)
    _cache["last_result"] = res
    out = np.empty((B, T, C), dtype=np.float32)
    for b in range(B):
        acc = res.results[2 * b]["out"].astype(np.float32) + \
            res.results[2 * b + 1]["out"].astype(np.float32)
        out[b] = acc.T
    return out


# revision 21
# speedup vs baseline: 1.0202x; 1.0202x over previous
"""Causal multi-head attention block (qkv proj + partial RoPE + causal attn +
out proj) for Trainium2, distributed over 8 NeuronCores.

Sharding: core i handles batch b = i//2 and head-group g = i%2 (6 of 12 heads).
Each core computes a partial output projection (contraction over its 6 heads'
384 channels); the host sums the two head-group partials per batch.

v4 design notes (over v3):
  - All matmul operands bf16 (separate pipelined LDWEIGHTS + FWL; fp32r
    fuses a serial 4-byte weight load into every matmul).
  - DMA coalescing: the Sync sequencer issues each DMA in ~600ns serially,
    and v3 had ~43 DMAs per tile (plus 25 at startup) — the issue stream
    alone stalled the pipeline.  v4: weights = 1 DMA each, x = 1/tile,
    rope scatter = 2/tile, pass scatter = 1/tile, rowsums = 3/tile,
    out = 1/tile.  Enabled by reordering wqk rows into (block, head-parity,
    dim) groups of 96 so every scatter is one affine access pattern.
  - Deferred finish: softmax normalization (rowsum DMA -> reciprocal ->
    broadcast matmul -> multiply) and the out-projection of tile jt are
    emitted AFTER tile jt+1's projection matmuls, so the serial chain
    never blocks the strict-FIFO PE queue (v3 lost ~6us per tile there).
  - PSUM->SBUF evictions of projection tiles moved to ScalarE (idle during
    the projection phase; DVE was becoming the second bottleneck).
  - Causal diag mask as one fused gpsimd op over both heads.
"""

import numpy as np

B, T, C = 4, 2048, 768
NH, HD, RD = 12, 64, 16
NHL = NH // 2          # heads per core (local)
NPAIR = NHL // 2       # head pairs per core
CL = NHL * HD          # local channels (384)
TQ = 512               # q tile
NTQ = T // TQ
NKT = T // 128         # k tiles of 128

_cache = {}


def _build():
    import concourse.bacc as bacc
    import concourse.mybir as mybir
    import concourse.tile as tile

    F32 = mybir.dt.float32
    BF16 = mybir.dt.bfloat16
    AF = mybir.ActivationFunctionType
    MUL = mybir.AluOpType.mult
    SUB = mybir.AluOpType.subtract
    ADD = mybir.AluOpType.add

    nc = bacc.Bacc(trn_type="TRN2", name="attn8")

    xt = nc.dram_tensor("xt", [C, T], BF16, kind="ExternalInput")
    wqkt = nc.dram_tensor("wqkt", [C, 2 * CL], BF16, kind="ExternalInput")
    wvt = nc.dram_tensor("wvt", [C, CL], BF16, kind="ExternalInput")
    wot = nc.dram_tensor("wot", [CL, C], BF16, kind="ExternalInput")
    cosb = nc.dram_tensor("cosb", [96, T], BF16, kind="ExternalInput")
    sinb = nc.dram_tensor("sinb", [96, T], BF16, kind="ExternalInput")
    tri = nc.dram_tensor("tri", [128, 2, 128], BF16, kind="ExternalInput")
    e6 = nc.dram_tensor("e6", [6, NPAIR * 128], BF16, kind="ExternalInput")
    out = nc.dram_tensor("out", [C, T], BF16, kind="ExternalOutput")

    # qk-projection M-tiles (wqkt column order, host-built), all 96 rows:
    #   tile 0: r1 rows (g, hh, d8)  g=block 0..5 (q p0..p2, k p0..p2),
    #           hh=head parity, d=rope dim 0:8
    #   tile 1: r2 rows (g, hh, d8)  rope dims 8:16
    #   tiles 2..7: pass tile g: (hh, d48) = block g's dims 16:64
    with tile.TileContext(nc) as tc:
        with (
            tc.tile_pool(name="persist", bufs=1) as pp,
            tc.tile_pool(name="weights", bufs=1) as wp,
            tc.tile_pool(name="xload", bufs=2) as xlp,
            tc.tile_pool(name="pstage", bufs=2) as psg,
            tc.tile_pool(name="ropet", bufs=1) as rtp,
            tc.tile_pool(name="expp", bufs=3) as xpp,
            tc.tile_pool(name="misc", bufs=2) as msc,
            tc.tile_pool(name="onorm", bufs=6) as onp,
            tc.tile_pool(name="rsp", bufs=2) as rsp,
            tc.tile_pool(name="flex", bufs=2, space="PSUM") as flx,
            tc.tile_pool(name="sps", bufs=2, space="PSUM") as sps,
            tc.tile_pool(name="ops", bufs=1, space="PSUM") as ops,
        ):
            qk_sb = pp.tile([128, 2 * NPAIR, T], BF16, tag="qk")
            v_sb = pp.tile([128, NKT, NHL, HD + 1], BF16, tag="v")
            o_sb = pp.tile([128, NPAIR, T], BF16, tag="o")
            cos_t = pp.tile([96, T], BF16, tag="cos")
            sin_t = pp.tile([96, T], BF16, tag="sin")
            tri_t = pp.tile([128, 2, 128], BF16, tag="tri")
            e6_t = pp.tile([6, NPAIR * 128], BF16, tag="e6")
            rot1 = pp.tile([96, T], BF16, tag="rot1")
            rot2 = pp.tile([96, T], BF16, tag="rot2")

            wqk_t = wp.tile([128, C // 128, 2 * CL], BF16, tag="wqk")
            wv_t = wp.tile([128, C // 128, CL], BF16, tag="wv")
            wo_t = wp.tile([128, NPAIR, C], BF16, tag="wo")

            qk_v = qk_sb.rearrange("(hh p) b t -> hh p b t", hh=2)

            # startup order: x(0) + rope weight columns first so the first
            # projection matmuls start as early as possible
            x_tiles = {}
            x_tiles[0] = xlp.tile([128, C // 128, TQ], BF16, tag="x", name="x0")
            nc.sync.dma_start(
                x_tiles[0],
                xt.rearrange("(co p) t -> p co t", p=128)[:, :, 0:TQ])
            wqk_r = wqkt.rearrange("(co p) m -> p co m", p=128)
            nc.sync.dma_start(wqk_t[:, :, 0:192], wqk_r[:, :, 0:192])
            nc.sync.dma_start(cos_t, cosb[:, :])
            nc.sync.dma_start(sin_t, sinb[:, :])
            nc.sync.dma_start(wqk_t[:, :, 192:2 * CL], wqk_r[:, :, 192:2 * CL])
            nc.sync.dma_start(wv_t, wvt.rearrange("(co p) m -> p co m", p=128))
            nc.sync.dma_start(tri_t, tri[:, :, :])
            nc.gpsimd.memset(
                v_sb.bitcast(mybir.dt.uint16).rearrange("p a b c -> p (a b c)"),
                0x3F80)  # bf16 1.0 bit pattern
            # needed first at finish(0), well after startup
            nc.sync.dma_start(wo_t, wot.rearrange("(po p) m -> p po m", p=128))
            nc.sync.dma_start(e6_t, e6[:, :])

            state = {}  # per-jt tiles needed by the deferred finish

            def proj(jt):
                ts = slice(jt * TQ, (jt + 1) * TQ)
                x_jt = x_tiles[jt]
                # rope M-tiles (r1, r2) into one 2-bank psum tile
                ps_r = sps.tile([128, 2, TQ], F32, tag="s")
                for mt in range(2):
                    for c in range(C // 128):
                        nc.tensor.matmul(
                            ps_r[0:96, mt, :],
                            wqk_t[:, c, 96 * mt:96 * (mt + 1)],
                            x_jt[:, c], start=(c == 0), stop=(c == C // 128 - 1))
                # rope: rot1 = r1*cos - r2*sin ; rot2 = r2*cos + r1*sin
                t1 = rtp.tile([96, TQ], F32, tag="t1")
                t2 = rtp.tile([96, TQ], F32, tag="t2")
                nc.vector.tensor_tensor(t1, ps_r[0:96, 0, :], cos_t[:, ts], MUL)
                nc.vector.tensor_tensor(t2, ps_r[0:96, 1, :], sin_t[:, ts], MUL)
                nc.vector.tensor_tensor(rot1[:, ts], t1, t2, SUB)
                t3 = rtp.tile([96, TQ], F32, tag="t1")
                t4 = rtp.tile([96, TQ], F32, tag="t2")
                nc.vector.tensor_tensor(t3, ps_r[0:96, 1, :], cos_t[:, ts], MUL)
                nc.vector.tensor_tensor(t4, ps_r[0:96, 0, :], sin_t[:, ts], MUL)
                nc.vector.tensor_tensor(rot2[:, ts], t3, t4, ADD)
                # rope scatter: rot rows are (hh, d, g), so the 48-row block of
                # one head parity linearizes exactly as the dst (d8, blk6, t)
                # iteration; one plain-slice DMA per (rot tile, head parity)
                for hh in range(2):
                    nc.sync.dma_start(qk_v[hh, 0:8, :, ts],
                                      rot1[48 * hh:48 * hh + 48, ts])
                    nc.sync.dma_start(qk_v[hh, 8:16, :, ts],
                                      rot2[48 * hh:48 * hh + 48, ts])

                # pass M-tiles: 6 tiles of 96 rows -> staged -> scatter DMA.
                # q tiles (g 0:3) first + their scatter issued early: the
                # attention phase needs q rows immediately, while k rows of
                # this tile are only needed by the 4 diagonal blocks at the
                # end of the kt loop.
                stg = psg.tile([96, C // 128, TQ], BF16, tag="pstg")
                for g in range(6):
                    ps = flx.tile([128, TQ], F32, tag="flex")
                    for c in range(C // 128):
                        nc.tensor.matmul(
                            ps[0:96], wqk_t[:, c, 192 + 96 * g:192 + 96 * (g + 1)],
                            x_jt[:, c], start=(c == 0), stop=(c == C // 128 - 1))
                    nc.scalar.copy(stg[:, g], ps[0:96])
                    if g == 2:
                        for hh in range(2):
                            nc.sync.dma_start(qk_v[hh, 16:64, 0:3, ts],
                                              stg[48 * hh:48 * hh + 48, 0:3, :])
                for hh in range(2):
                    nc.sync.dma_start(qk_v[hh, 16:64, 3:6, ts],
                                      stg[48 * hh:48 * hh + 48, 3:6, :])

                # v projection (x stationary -> [token, channel] layout)
                for vt in range(TQ // 128):
                    pvf = flx.tile([128, TQ], F32, tag="flex")
                    pv = pvf[:, 0:CL]
                    kt0 = jt * (TQ // 128) + vt
                    for c in range(C // 128):
                        nc.tensor.matmul(
                            pv, x_jt[:, c, vt * 128:(vt + 1) * 128],
                            wv_t[:, c], start=(c == 0), stop=(c == C // 128 - 1))
                    nc.scalar.copy(
                        v_sb[:, kt0, :, 0:HD],
                        pv.rearrange("p (h d) -> p h d", d=HD))

            def attention(jq):
                qs = slice(jq * TQ, (jq + 1) * TQ)
                rs6 = rsp.tile([6, TQ], F32, tag="rs6")
                ouns = []
                for p in range(NPAIR):
                    qb = qk_sb[:, p, qs]
                    kb = qk_sb[:, NPAIR + p, :]
                    o_ps = ops.tile([128, 2, TQ], F32, tag="o")
                    nkt = 4 * (jq + 1)
                    for kt in range(nkt):
                        m = kt - 4 * jq
                        a = 0 if m < 0 else 128 * m
                        ks = slice(kt * 128, (kt + 1) * 128)
                        sg = sps.tile([128, 2, TQ], F32, tag="s")
                        nc.tensor.matmul(
                            sg[:, 0, a:TQ], kb[0:64, ks], qb[0:64, a:TQ],
                            start=True, stop=True, tile_position=(0, 0))
                        nc.tensor.matmul(
                            sg[:, 1, a:TQ], kb[64:128, ks], qb[64:128, a:TQ],
                            start=True, stop=True, tile_position=(64, 0))
                        ep = xpp.tile([128, 2, TQ], BF16, tag="e")
                        nc.scalar.activation(ep[:, :, a:TQ], sg[:, :, a:TQ],
                                             AF.Exp, scale=0.125)
                        if m >= 0:
                            nc.gpsimd.tensor_tensor(
                                ep[:, :, a:a + 128],
                                ep[:, :, a:a + 128], tri_t, MUL)
                        for h in range(2):
                            nc.tensor.matmul(
                                o_ps[0:65, h, a:TQ],
                                v_sb[:, kt, 2 * p + h, :], ep[:, h, a:TQ],
                                start=(kt == 0), stop=(kt == nkt - 1))
                    oun = onp.tile([128, 2, TQ], F32, tag="oun")
                    nc.vector.tensor_copy(oun[0:65, :, :], o_ps[0:65, :, :])
                    for h in range(2):
                        nc.sync.dma_start(rs6[2 * p + h:2 * p + h + 1, :],
                                          oun[64:65, h, :])
                    ouns.append(oun)
                state[jq] = (rs6, ouns)

            def finish_recip(jq):
                # 1/x as exp(-ln(x)) on ACT: ~1.4us vs 3.3us for the DVE
                # iterative divide (6 lanes only), same table set as the
                # attention exp, and it keeps the boundary chain off DVE
                rs6, ouns = state[jq]
                ln6 = rsp.tile([6, TQ], F32, tag="ln6")
                rinv6 = rsp.tile([6, TQ], BF16, tag="rinv6")
                nc.scalar.activation(ln6, rs6, AF.Ln)
                nc.scalar.activation(rinv6, ln6, AF.Exp, scale=-1.0)
                state[jq] = (rinv6, ouns)

            def finish(jq):
                qs = slice(jq * TQ, (jq + 1) * TQ)
                rinv6, ouns = state.pop(jq)
                for p in range(NPAIR):
                    bc = flx.tile([128, TQ], F32, tag="flex")
                    nc.tensor.matmul(bc, e6_t[:, p * 128:(p + 1) * 128], rinv6,
                                     start=True, stop=True)
                    oun = ouns[p]
                    with nc.allow_low_precision(reason="o normalize in bf16"):
                        nc.vector.tensor_tensor(
                            o_sb[0:64, p, qs], oun[0:64, 0, :], bc[0:64], MUL)
                        nc.vector.tensor_tensor(
                            o_sb[64:128, p, qs], oun[0:64, 1, :], bc[64:128], MUL)
                ost = msc.tile([128, C // 128, TQ], BF16, tag="ost")
                for dt in range(C // 128):
                    po = flx.tile([128, TQ], F32, tag="flex")
                    for p in range(NPAIR):
                        nc.tensor.matmul(
                            po, wo_t[:, p, dt * 128:(dt + 1) * 128],
                            o_sb[:, p, qs], start=(p == 0), stop=(p == NPAIR - 1))
                    nc.vector.tensor_copy(ost[:, dt], po)
                nc.sync.dma_start(
                    out.rearrange("(do p) t -> p do t", p=128)[:, :, qs], ost)

            for jt in range(NTQ):
                if jt >= 1:
                    finish_recip(jt - 1)
                proj(jt)
                if jt + 1 < NTQ:
                    x_tiles[jt + 1] = xlp.tile(
                        [128, C // 128, TQ], BF16, tag="x", name=f"x{jt + 1}")
                    nc.sync.dma_start(
                        x_tiles[jt + 1],
                        xt.rearrange("(co p) t -> p co t", p=128)[
                            :, :, (jt + 1) * TQ:(jt + 2) * TQ])
                attention(jt)
                # finish(jt-1) emitted after attention(jt): its reciprocal
                # chain then has the whole attention phase to complete, so
                # the broadcast matmul never stalls the PE queue
                if jt >= 1:
                    finish(jt - 1)
            finish_recip(NTQ - 1)
            finish(NTQ - 1)

    nc.compile()
    return nc


def _host_inputs(x, w_qkv, w_out):
    """Build per-core input dicts. Core i: batch i//2, head-group i%2."""
    import ml_dtypes

    BF = ml_dtypes.bfloat16
    xf = np.ascontiguousarray(x, dtype=np.float32)
    w3 = np.asarray(w_qkv, dtype=np.float32).reshape(3, NH, HD, C)
    wo = np.asarray(w_out, dtype=np.float32)

    per_group = []
    for g in range(2):
        h0 = g * NHL
        # rope tiles r1/r2 rows: (hh, d8, blk); blk 0..2 = q pairs, 3..5 = k
        r1, r2, passes = [], [], []
        for hh in range(2):
            for d in range(8):
                for blk in range(6):
                    tn = 0 if blk < 3 else 1
                    pr = blk % 3
                    h = h0 + 2 * pr + hh
                    r1.append(w3[tn, h, d:d + 1])
                    r2.append(w3[tn, h, 8 + d:9 + d])
        for blk in range(6):
            tn = 0 if blk < 3 else 1
            pr = blk % 3
            rows = []
            for hh in range(2):
                h = h0 + 2 * pr + hh
                rows.append(w3[tn, h, 16:64])
            passes.append(np.concatenate(rows, axis=0))    # [96, C]
        wqk = np.concatenate(
            [np.concatenate(r1, axis=0), np.concatenate(r2, axis=0)] + passes,
            axis=0)                                        # [768, C]
        wqkt = np.ascontiguousarray(wqk.T).astype(BF)      # [C, 768]
        wv = w3[2, h0:h0 + NHL].reshape(CL, C)             # [384, C]
        wvt = np.ascontiguousarray(wv.T).astype(BF)
        wotr = np.ascontiguousarray(wo[:, g * CL:(g + 1) * CL].T).astype(BF)
        per_group.append((wqkt, wvt, wotr))

    j = np.arange(RD // 2, dtype=np.float64)
    freqs = 1.0 / (10000.0 ** (2 * j / RD))
    t = np.arange(T, dtype=np.float64)
    ang = t[None, :] * freqs[:, None]                      # [8, T]
    # rows (hh, d, g): frequency d repeated for the 6 blocks, twice
    cosb = np.tile(np.repeat(np.cos(ang), 6, axis=0), (2, 1)).astype(BF)
    sinb = np.tile(np.repeat(np.sin(ang), 6, axis=0), (2, 1)).astype(BF)

    kk = np.arange(128)[:, None]
    qq = np.arange(128)[None, :]
    tri = np.broadcast_to(
        (kk <= qq)[:, None, :], (128, 2, 128)).astype(BF).copy()
    e6 = np.zeros((6, NPAIR * 128), dtype=np.float32)
    for p in range(NPAIR):
        e6[2 * p, p * 128:p * 128 + 64] = 1.0
        e6[2 * p + 1, p * 128 + 64:(p + 1) * 128] = 1.0
    e6 = e6.astype(BF)

    in_maps = []
    for i in range(8):
        b, g = divmod(i, 2)
        wqkt, wvt, wotr = per_group[g]
        in_maps.append({
            "xt": np.ascontiguousarray(xf[b].T).astype(BF),
            "wqkt": wqkt, "wvt": wvt, "wot": wotr,
            "cosb": cosb, "sinb": sinb, "tri": tri, "e6": e6,
        })
    return in_maps


def kernel(x, w_qkv, w_out, _trace=False):
    from concourse.bass_utils import run_bass_kernel_spmd

    if "nc" not in _cache:
        _cache["nc"] = _build()
    nc = _cache["nc"]
    in_maps = _host_inputs(x, w_qkv, w_out)
    res = run_bass_kernel_spmd(nc, in_maps, core_ids=list(range(8)),
                               trace=_trace)
    _cache["last_result"] = res
    out = np.empty((B, T, C), dtype=np.float32)
    for b in range(B):
        acc = res.results[2 * b]["out"].astype(np.float32) + \
            res.results[2 * b + 1]["out"].astype(np.float32)
        out[b] = acc.T
    return out
